# revision 61
# baseline (speedup 1.0000x reference)
"""Bass/Trainium2 kernel for masked (padding) multi-head self-attention.

Problem: B=2, T=2048, C=1024, H=16 heads of DH=64.
  q/k/v = x @ W* + b*  ->  att = softmax(mask(q k^T / 8))  ->  y = att @ v

Sharding over 8 NeuronCores: core = (batch b, head-group hg) with
b = core // 4, hg = core % 4; each core computes 4 heads for one batch
element (its [T, 256] slice of q/k/v from the Wq/Wk/Wv column slice).

Host-side preprocessing (inside kernel()):
  - Only valid (mask==1) tokens are gathered; the k-dim is padded to tp
    (multiple of 128 for PE k-tiles), the q/free dim trimmed to
    tq = nch*cw >= max valid (cw a multiple of 8: fp32r matmuls reject
    odd free sizes, s3d3_mm_fp32r_restrictions).
  - x ships as fp8 residual-split planes xh=e4m3(x), xl=e5m2(x-xh);
    each W ships as hi/lo planes of 16*W (power-of-2 prescale keeps the
    hi plane in e4m3 normal range), pre-swizzled to [d-half, partition,
    c-tile, 128] and packed 4-planes-per-uint8-container so each
    critical DMA is one contiguous transfer per partition row.

Device compute (per core), dtype/layout choices from an error study
(split-fp8 proj + bf16 e/v/out ~ 5e-3 metric vs the 2e-2 gate):
  qT/kT/v: 3-term DoubleRow fp8 matmuls (xh*Wh + xl*Wh + xh*Wl) over
    c-tile pairs at 0.5 cycles/row -- 25% cheaper than bf16 with ~2x
    better accuracy (effective ~12-bit mantissa).  qT/kT evict to f32r.
  sT[k,q] = sum_d kT[d,k] qT[d,q]   (f32r x f32r, 1.0 c/row at cw>=256)
  e = exp(s_raw/(8*256) + ebias_t)  (ACT; bias column kills pad k-rows;
    e stored bf16)
  y[q,dd] = sum_k e[k,q] vaug[k,dd] (lhsT=e stationary, rhs=v bf16
    moving, out [q-subtile, 65] accumulated over all k in PSUM; column
    64 of vaug is ones -> softmax denominator).  y staged bf16.
Normalization (numer/denom/16) and scatter back to [T, C] on host.

Schedule (TimelineSim 58083 ns/core vs 77354 baseline; HW-verified
rel err 5.1e-3):
  head ~13.4us: DMA-roofline on Wq/Wk-d0 + x planes (2.9MB); the
    d-tile-0 q projection + k chunk-0 chase the per-ct-pair transfers
    ct-major with 6 open PSUM groups; evictions alternate ACT/DVE.
  stream ~39.7us: ACT-bound, 36 exps near-dense.  Heads 0/1 sweep as a
    pair (PE-heavy phase: remaining k-d0/d1/v units drip in as budgeted
    fillers); heads 2 and 3 sweep singly so e(2,*) completes early and
    av(2) streams during head-3's exps.  AV runs in flipped orientation
    (out [q,65], 65-cycle instructions, no SBUF accumulator chain);
    per-chunk out DMAs overlap the stream.
  tail ~5.0us: head-3's final score tile runs through the ops pool as 3
    per-chunk exps (frees all sps banks one slot early); its AV chunks
    share one PSUM tile per chunk (independent accumulation regions,
    skip_group_check) for single evictions and only two out-DMAs
    (HWDGE generation, ~700ns/DMA, is the tail bottleneck).
Known-negative experiments (reverted): chunk-oriented av3 A/B k-splits
(ops-slot congestion); per-sub tail DMAs and ACT-queue out-DMAs (DGE
overhead/exp-slot theft); eager fillers beyond ~1.3us/slot budgets.
"""

import math
import sys

sys.path.insert(0, "/opt/trn_rl_repo")

import ml_dtypes
import numpy as np

import concourse.bacc as bacc
import concourse.mybir as mybir
import concourse.tile as tile
from concourse import bass_utils

F32 = mybir.dt.float32
F32R = mybir.dt.float32r
BF16 = mybir.dt.bfloat16
F8H = mybir.dt.float8e4
F8L = mybir.dt.float8e5
DR = mybir.MatmulPerfMode.DoubleRow
AF = mybir.ActivationFunctionType
NPBF = ml_dtypes.bfloat16
NP8H = ml_dtypes.float8_e4m3
NP8L = ml_dtypes.float8_e5m2
WS = 16.0  # power-of-2 prescale keeping fp8 W planes in normal range

B, T, C, H = 2, 2048, 1024, 16
DH = C // H            # 64
HPC = 4                # heads per core
CSL = HPC * DH         # 256, per-core column slice of C
N_CORES = 8
NCT = C // 128         # 8 contraction tiles over C

_CACHE: dict = {}


def _pick_dims(max_valid: int):
    """k-dim tiles (nkt, tp) and q-dim chunks (nch, cw, tq)."""
    mt = max(max_valid, 1)
    nkt = max(2, math.ceil(mt / 128))
    tp = nkt * 128
    nch = max(1, math.ceil(mt / 512))
    if nch < 3 and nch * 512 < tp:
        nch = min(3, math.ceil(tp / 512))
    # fp32r matmuls reject odd free sizes (s3d3_mm_fp32r_restrictions):
    # keep chunk widths a multiple of 8
    cw = min(512, math.ceil(mt / nch / 8) * 8)
    while nch * cw < mt:
        cw = min(512, cw + 8)
        if nch * cw < mt and cw == 512:
            nch += 1
    tq = nch * cw
    return tp, nkt, cw, nch, tq


def _subtiles(cw: int):
    offs, widths = [], []
    o = 0
    while o < cw:
        w = min(128, cw - o)
        offs.append(o)
        widths.append(w)
        o += w
    return list(zip(offs, widths))


def _build(tp, nkt, cw, nch, tq, with_bias):
    nc = bacc.Bacc("TRN2", target_bir_lowering=False, debug=False,
                   num_devices=N_CORES)

    # x and W ship as fp8 residual-split planes (hi=e4m3, lo=e5m2);
    # projections run as 3-term DoubleRow matmuls (hi*hi + lo*hi + hi*lo)
    # at 0.5 cycles/row -- 25% cheaper than bf16 with ~2x less error.
    # W planes are pre-swizzled [d-half, partition, c-tile, 128] so every
    # half-DMA is contiguous per partition row (no 256B-piece penalty).
    xh_d = nc.dram_tensor("xh", [C, tp], F8H, kind="ExternalInput")
    xl_d = nc.dram_tensor("xl", [C, tp], F8L, kind="ExternalInput")
    # qh/ql/kh/kl planes packed per d-half into one uint8 container so
    # each is a single contiguous DMA; slices are bitcast at use sites
    w0_d = nc.dram_tensor("w0", [128, 4, NCT, 128], mybir.dt.uint8,
                          kind="ExternalInput")
    w1_d = nc.dram_tensor("w1", [128, 4, NCT, 128], mybir.dt.uint8,
                          kind="ExternalInput")
    wv_d = nc.dram_tensor("wv", [128, 2, NCT, CSL], mybir.dt.uint8,
                          kind="ExternalInput")
    # misc: col 0..nkt-1 = ebias per k-tile; col nkt..nkt+3 = bq/bk halves
    nmc = nkt + (4 if with_bias else 0)
    misc_d = nc.dram_tensor("misc", [128, nmc], F32, kind="ExternalInput")
    onesv_d = nc.dram_tensor("onesv", [128, nkt * HPC], BF16,
                             kind="ExternalInput")
    if with_bias:
        bv_d = nc.dram_tensor("bv", [1, CSL], F32, kind="ExternalInput")
    subs = _subtiles(cw)
    ns = len(subs)
    out_d = nc.dram_tensor("out", [128, nch, ns, HPC, DH + 1], BF16,
                           kind="ExternalOutput")
    # head-3 leaves in [dd, chunk] orientation (tail-optimized path)
    out3_d = nc.dram_tensor("out3", [DH + 1, nch, cw], F32,
                            kind="ExternalOutput")
    import os
    _dbg = bool(os.environ.get("KERNEL_DEBUG"))
    if _dbg:
        dbg_d = nc.dram_tensor("dbg", [128, 2, tp], F32,
                               kind="ExternalOutput")

    chunks = [(j * cw, cw) for j in range(nch)]
    seq_heads = nkt >= 12          # SBUF can't hold 4 heads of e-tiles

    with tile.TileContext(nc) as tc:
        with tc.tile_pool(name="const", bufs=1) as cp:
            xh_sb = cp.tile([128, NCT, tp], F8H, tag="xh")
            xl_sb = cp.tile([128, NCT, tp], F8L, tag="xl")
            w01_sb = [cp.tile([128, 4, NCT, 128], mybir.dt.uint8,
                              tag=f"w{p}", name=f"w{p}") for p in range(2)]
            wv_sb = cp.tile([128, 2, NCT, CSL], mybir.dt.uint8, tag="wv")
            misc_sb = cp.tile([128, nmc], F32, tag="misc")
            qt_sb = [cp.tile([128, tq], F32R, tag=f"qt{p}", name=f"qt{p}")
                     for p in range(2)]
            kt_sb = [cp.tile([128, tp], F32R, tag=f"kt{p}", name=f"kt{p}")
                     for p in range(2)]
            v_sb = cp.tile([128, nkt, HPC, DH + 1], BF16, tag="v")
            y_sb = cp.tile([128, nch, ns, HPC, DH + 1], BF16, tag="y")
            y3_sb = cp.tile([DH + 1, nch, cw], F32, tag="y3")
            ebias_sb = misc_sb[:, 0:nkt]
            if with_bias:
                bqk_sb = misc_sb[:, nkt:nkt + 4]
                bv_sb = cp.tile([1, CSL], F32R, tag="bv")
                ones_sb = cp.tile([1, 128], F32R, tag="ones")

            scratch = cp.tile([1, 8], F32, tag="scratch")

            xh_r = xh_d.ap().rearrange("(i p) t -> p i t", p=128)
            xl_r = xl_d.ap().rearrange("(i p) t -> p i t", p=128)
            # critical-path DMAs in strict SP-queue order: Wq/Wk d0 plane
            # halves, the x hi/lo streams (d-tile-0 projection chases them
            # per ct-pair), then d1 halves and Wv off the critical path.
            nc.sync.dma_start(w01_sb[0][:], w0_d.ap()[:])
            nc.sync.dma_start(misc_sb[:], misc_d.ap()[:])
            for i in range(0, NCT, 2):
                nc.sync.dma_start(xh_sb[:, i:i + 2, :], xh_r[:, i:i + 2, :])
                nc.sync.dma_start(xl_sb[:, i:i + 2, :], xl_r[:, i:i + 2, :])
            nc.sync.dma_start(w01_sb[1][:], w1_d.ap()[:])
            nc.sync.dma_start(wv_sb[:], wv_d.ap()[:])
            if with_bias:
                nc.sync.dma_start(bv_sb[:], bv_d.ap()[:].bitcast(F32R))
                nc.gpsimd.memset(ones_sb[:], 1.0)

            # denominator ones-column of vaug; zero the kT columns beyond
            # the projected range (pad k-tokens; killed by ebias anyway but
            # must be finite)
            nc.sync.dma_start(
                v_sb[:, :, :, DH],
                onesv_d.ap().rearrange("p (t h) -> p t h", h=HPC))
            if tq < tp:
                nc.gpsimd.memset(kt_sb[0][:, tq:tp].bitcast(F32), 0.0)
                nc.gpsimd.memset(kt_sb[1][:, tq:tp].bitcast(F32), 0.0)

            # warm the ACT exp table during the DMA window
            nc.gpsimd.memset(scratch[:], 0.0)
            nc.scalar.activation(scratch[:], scratch[:], AF.Exp)

            def evict_qk(o_ap, ps_ap, bcol, alt=1):
                # PSUM reads: DVE/ACT only (GPSIMD cannot access PSUM);
                # alternating engines halves the eviction chain on the
                # critical path out of phase A.
                if with_bias:
                    if alt % 2 == 0:
                        nc.scalar.activation(o_ap, ps_ap, AF.Identity,
                                             bias=bqk_sb[:, bcol:bcol + 1])
                    else:
                        nc.vector.tensor_scalar_add(o_ap, ps_ap,
                                                    bqk_sb[:, bcol:bcol + 1])
                else:
                    if alt % 2 == 0:
                        nc.scalar.copy(o_ap, ps_ap)
                    else:
                        nc.vector.tensor_copy(o_ap, ps_ap)

            NPAIR = NCT // 2
            QK_TERMS = (0, 1)  # matrix index: 0 = q, 1 = k

            def qk_terms(mi, p, cts):
                wt = w01_sb[p]
                wh = wt[:, 2 * mi, cts, :].bitcast(F8H)
                wl = wt[:, 2 * mi + 1, cts, :].bitcast(F8L)
                return ((wh, xh_sb), (wh, xl_sb), (wl, xh_sb))

            def proj_chunks(pool, tag, p, work):
                # ct-pair-major emission with the accumulation groups open
                # so the DoubleRow matmuls chase the x-plane DMAs; work
                # items are (w_pair, o_sb, bias-col-base, chunk-off, w).
                tiles = [pool.tile([128, cw], F32, tag=tag, name="pqk")
                         for _ in work]
                for cp_i in range(NPAIR):
                    cts = slice(2 * cp_i, 2 * cp_i + 2)
                    for ps, (w_pair, o_sb, bc, off, w) in zip(tiles, work):
                        for ti, (lhs, x_sb) in enumerate(
                                qk_terms(w_pair, p, cts)):
                            nc.tensor.matmul(
                                ps[:, 0:w],
                                lhs,
                                x_sb[:, cts, off:off + w],
                                start=(cp_i == 0 and ti == 0),
                                stop=(cp_i == NPAIR - 1 and ti == 2),
                                perf_mode=DR,
                            )
                for n, (ps, (w_pair, o_sb, bc, off, w)) in enumerate(
                        zip(tiles, work)):
                    evict_qk(o_sb[p][:, off:off + w], ps[:, 0:w], bc + p, n)


            # phase A: qkT d-tile-0 projection with 6 psum slots so all six
            # accumulation groups pipeline with the incoming xt DMAs.
            with tc.tile_pool(name="pa", bufs=6, space="PSUM") as pa:
                # warm the PE (HAM clock gate) during the DMA window
                wsc = cp.tile([128, 16], F32, tag="wsc")
                nc.gpsimd.memset(wsc[:], 0.0)
                for _ in range(60):
                    wps = pa.tile([16, 16], F32, tag="a", name="wps")
                    nc.tensor.matmul(wps[:], wsc[:, 0:16], wsc[:],
                                     start=True, stop=True)
                # q d0 all chunks + k d0 chunk 0 only: 4 matmuls per ct
                # keeps the chase under the per-tile DMA time; k d0 ch1/2
                # run as early main-loop units (first needed at t=3).
                proj_chunks(pa, "a", 0,
                            [(QK_TERMS[0], qt_sb, 0, off, w)
                             for off, w in chunks]
                            + [(QK_TERMS[1], kt_sb, 2, chunks[0][0],
                                chunks[0][1])])

            ebufs = (nkt + 3) if seq_heads else (4 * nkt + 2)

            with (
                tc.tile_pool(name="ops", bufs=2, space="PSUM") as ops,
                tc.tile_pool(name="epool", bufs=ebufs) as ep,
            ):
                e_tiles: dict = {}
                chunk_cnt: dict = {}

                def note_evict(h, j, si=None):
                    c = chunk_cnt.get((h, j), 0) + 1
                    chunk_cnt[(h, j)] = c
                    if c == ns:
                        nc.sync.dma_start(out_d.ap()[:, j, :, h, :],
                                          y_sb[:, j, :, h, :])

                def proj_v_unit(t):
                    ps = ops.tile([128, CSL], F32, tag="o", name="pv")
                    tsl = slice(t * 128, (t + 1) * 128)
                    for cp_i in range(NPAIR):
                        cts = slice(2 * cp_i, 2 * cp_i + 2)
                        wvh = wv_sb[:, 0, cts, :].bitcast(F8H)
                        wvl = wv_sb[:, 1, cts, :].bitcast(F8L)
                        terms = ((xh_sb[:, cts, tsl], wvh),
                                 (xl_sb[:, cts, tsl], wvh),
                                 (xh_sb[:, cts, tsl], wvl))
                        for ti, (xs, wvs) in enumerate(terms):
                            nc.tensor.matmul(
                                ps[:],
                                xs,
                                wvs,
                                start=(cp_i == 0 and ti == 0),
                                stop=(not with_bias
                                      and cp_i == NPAIR - 1 and ti == 2),
                                perf_mode=DR,
                            )
                    if with_bias:
                        nc.tensor.matmul(ps[:], ones_sb[:], bv_sb[:],
                                         start=False, stop=True)
                    nc.vector.tensor_copy(
                        v_sb[:, t, :, 0:DH],
                        ps[:].rearrange("p (h d) -> p h d", h=HPC),
                    )

                def qkd1_unit(w_pair, o_sb, bc, off, w, n):
                    ps = ops.tile([128, cw], F32, tag="o", name="pqk1")
                    for cp_i in range(NPAIR):
                        cts = slice(2 * cp_i, 2 * cp_i + 2)
                        for ti, (lhs, x_sb) in enumerate(
                                qk_terms(w_pair, 1, cts)):
                            nc.tensor.matmul(
                                ps[:, 0:w],
                                lhs,
                                x_sb[:, cts, off:off + w],
                                start=(cp_i == 0 and ti == 0),
                                stop=(cp_i == NPAIR - 1 and ti == 2),
                                perf_mode=DR,
                            )
                    evict_qk(o_sb[1][:, off:off + w], ps[:, 0:w], bc + 1)

                def scores(sps_pool, h, t, filler=None, split_exp=False):
                    pd, po = h // 2, (h % 2) * 64
                    qt_h, kt_h = qt_sb[pd], kt_sb[pd]
                    ps = sps_pool.tile([128, nch, 512], F32, tag="s",
                                       name="sps")
                    for j, (off, w) in enumerate(chunks):
                        nc.tensor.matmul(
                            ps[:, j, 0:w],
                            kt_h[po:po + 64, t * 128:(t + 1) * 128],
                            qt_h[po:po + 64, off:off + w],
                            start=True, stop=True,
                        )
                    if filler:
                        filler(t)
                    e_t = ep.tile([128, nch, cw], BF16, tag="e", name="e")
                    if split_exp:
                        # per-chunk exps let the final AV/evict/DMA chain
                        # pipeline chunk-by-chunk behind the last exp
                        for j in range(nch):
                            nc.scalar.activation(
                                e_t[:, j, :], ps[:, j, 0:cw], AF.Exp,
                                bias=ebias_sb[:, t:t + 1],
                                scale=0.125 / (WS * WS),
                            )
                    else:
                        nc.scalar.activation(
                            e_t[:], ps[:, :, 0:cw], AF.Exp,
                            bias=ebias_sb[:, t:t + 1],
                            scale=0.125 / (WS * WS),
                        )
                    e_tiles[(h, t)] = e_t

                def scores_pair(sps_pool, hA, hB, t, filler=None):
                    # hA/hB share a qT/kT d-tile at partition offsets 0/64;
                    # alternating the chunk matmuls lets the PE row-groups
                    # overlap the two heads' streams.
                    pd = hA // 2
                    qt_h, kt_h = qt_sb[pd], kt_sb[pd]
                    pss = {}
                    for h in (hA, hB):
                        pss[h] = sps_pool.tile([128, nch, 512], F32, tag="s",
                                               name="sps")
                    for j, (off, w) in enumerate(chunks):
                        for h in (hA, hB):
                            po = (h % 2) * 64
                            nc.tensor.matmul(
                                pss[h][:, j, 0:w],
                                kt_h[po:po + 64, t * 128:(t + 1) * 128],
                                qt_h[po:po + 64, off:off + w],
                                start=True, stop=True,
                            )
                    if filler:
                        filler(t)
                    for h in (hA, hB):
                        e_t = ep.tile([128, nch, cw], BF16, tag="e", name="e")
                        nc.scalar.activation(
                            e_t[:], pss[h][:, :, 0:cw], AF.Exp,
                            bias=ebias_sb[:, t:t + 1],
                            scale=0.125 / (WS * WS),
                        )
                        e_tiles[(h, t)] = e_t

                def scores_last(h, t):
                    # final tile of the last head: per-chunk psums from the
                    # ops pool + per-chunk exps.  The sps banks are all
                    # free one slot earlier, so the tail AV groups
                    # pre-accumulate, and AV/evict/DMA pipeline per chunk
                    # behind the three chunk-exps.
                    pd, po = h // 2, (h % 2) * 64
                    qt_h, kt_h = qt_sb[pd], kt_sb[pd]
                    e_t = ep.tile([128, nch, cw], BF16, tag="e", name="e")
                    for j, (off, w) in enumerate(chunks):
                        ps = ops.tile([128, cw], F32, tag="o", name="sl")
                        nc.tensor.matmul(
                            ps[:, 0:w],
                            kt_h[po:po + 64, t * 128:(t + 1) * 128],
                            qt_h[po:po + 64, off:off + w],
                            start=True, stop=True,
                        )
                        nc.scalar.activation(
                            e_t[:, j, :], ps[:, 0:cw], AF.Exp,
                            bias=ebias_sb[:, t:t + 1],
                            scale=0.125 / (WS * WS),
                        )
                    e_tiles[(h, t)] = e_t

                def av_sub(pool, h, j, s_off, s_w, si, act_evict=False,
                           ts=None, accum=False, note=True):
                    if ts is None:
                        ts = range(nkt)
                    avp = pool.tile([128, DH + 1], F32, tag="o", name="av")
                    for i, t in enumerate(ts):
                        nc.tensor.matmul(
                            avp[0:s_w, :],
                            e_tiles[(h, t)][:, j, s_off:s_off + s_w],
                            v_sb[:, t, h, :],
                            start=(i == 0), stop=(i == len(ts) - 1),
                        )
                    if accum:
                        nc.vector.tensor_add(
                            y_sb[0:s_w, j, si, h, :],
                            y_sb[0:s_w, j, si, h, :], avp[0:s_w, :])
                    elif act_evict:
                        nc.scalar.copy(y_sb[0:s_w, j, si, h, :], avp[0:s_w, :])
                    else:
                        nc.vector.tensor_copy(
                            y_sb[0:s_w, j, si, h, :], avp[0:s_w, :])
                    if note:
                        note_evict(h, j, si)

                if seq_heads:
                    with tc.tile_pool(name="sps", bufs=2,
                                      space="PSUM") as sps_pool:
                        for off, w in chunks[1:]:
                            proj_chunks(ops, "o", 0,
                                        [(QK_TERMS[1], kt_sb, 2, off, w)])
                        proj_chunks(ops, "o", 1,
                                    [(QK_TERMS[0], qt_sb, 0, off, w)
                                     for off, w in chunks]
                                    + [(QK_TERMS[1], kt_sb, 2, off, w)
                                       for off, w in chunks])
                        for t in range(nkt):
                            proj_v_unit(t)
                        for h in range(HPC):
                            for t in range(nkt):
                                scores(sps_pool, h, t)
                            for j in range(nch):
                                for si, (o, w) in enumerate(subs):
                                    av_sub(ops, h, j, o, w, si)
                else:
                    # fillers for the pair(0,1) sweep: v tiles + the
                    # d-tile-1 q/k projection, one unit per exp-slot; the
                    # overflow drains into the later single-head sweeps
                    # where the PE is otherwise starved.
                    units = [("k0", (off, w)) for off, w in chunks[1:]]
                    units += [("v", t) for t in range(nkt)]
                    n = 0
                    for w_pair, o_sb, bc in ((QK_TERMS[0], qt_sb, 0),
                                             (QK_TERMS[1], kt_sb, 2)):
                        for off, w in chunks:
                            units.insert(len(chunks) - 1 + 2 * n + 1,
                                         ("d1", (w_pair, o_sb, bc, off, w,
                                                 n)))
                            n += 1

                    def emit_unit(units):
                        if not units:
                            return False
                        kind, a = units.pop(0)
                        if kind == "v":
                            proj_v_unit(a)
                        elif kind == "k0":
                            proj_chunks(ops, "o", 0,
                                        [(QK_TERMS[1], kt_sb, 2, a[0],
                                          a[1])])
                        else:
                            qkd1_unit(*a)
                        return True

                    def subwork(h):
                        return [(h, j, si, o, w) for j in range(nch)
                                for si, (o, w) in enumerate(subs)]

                    av01 = subwork(0) + subwork(1)
                    av2 = subwork(2)
                    av3 = subwork(3)
                    split3 = False
                    ka3 = list(range(nkt - 3))
                    kb3 = list(range(nkt - 3, nkt))

                    def av3_chunk(pool, j, ts, accum):
                        # old-orientation AV for the tail head: out
                        # [dd, chunk] costs more PE but only nch groups,
                        # each finishing 144ns after its last e-tile.
                        p3 = pool.tile([DH + 1, cw], F32, tag="o", name="av3")
                        for i, t in enumerate(ts):
                            nc.tensor.matmul(
                                p3[:],
                                v_sb[:, t, HPC - 1, :],
                                e_tiles[(HPC - 1, t)][:, j, :],
                                start=(i == 0), stop=(i == len(ts) - 1),
                            )
                        if accum:
                            nc.vector.tensor_add(y3_sb[:, j, :],
                                                 y3_sb[:, j, :], p3[:])
                            nc.sync.dma_start(out3_d.ap()[:, j, :],
                                              y3_sb[:, j, :])
                        else:
                            nc.vector.tensor_copy(y3_sb[:, j, :], p3[:])

                    with tc.tile_pool(name="sps", bufs=2,
                                      space="PSUM") as sps_pool:
                        def filler01(t):
                            if t is not None and t < 1:
                                return
                            budget = 1300
                            while budget > 0 and units:
                                emit_unit(units)
                                budget -= 800

                        # heads 0/1 paired (PE-heavy phase), then heads 2
                        # and 3 swept singly: e(2,*) completes a full sweep
                        # early, so av(2) streams during head-3's exps and
                        # only av(3) remains after the last exp.
                        for t in range(nkt):
                            scores_pair(sps_pool, 0, 1, t, filler=filler01)

                        def filler2(t):
                            budget = 500
                            while budget > 0:
                                if units:
                                    emit_unit(units)
                                    budget -= 800
                                elif av01:
                                    h, j, si, o, w = av01.pop(0)
                                    av_sub(ops, h, j, o, w, si)
                                    budget -= 260
                                else:
                                    return

                        for t in range(nkt):
                            scores(sps_pool, 2, t, filler=filler2)
                        while units:
                            emit_unit(units)

                        def filler3(t):
                            budget = 500
                            while budget > 0:
                                if units:
                                    emit_unit(units)
                                    budget -= 800
                                elif av01:
                                    h, j, si, o, w = av01.pop(0)
                                    av_sub(ops, h, j, o, w, si)
                                    budget -= 260
                                elif av2:
                                    h, j, si, o, w = av2.pop(0)
                                    av_sub(ops, h, j, o, w, si)
                                    budget -= 260
                                else:
                                    return

                        for t in range(nkt - 1):
                            scores(sps_pool, 3, t, filler=filler3)
                        scores_last(3, nkt - 1)
                        while av01:
                            h, j, si, o, w = av01.pop(0)
                            av_sub(ops, h, j, o, w, si)
                        while av2:
                            h, j, si, o, w = av2.pop(0)
                            av_sub(ops, h, j, o, w, si)

                    if _dbg:
                        nc.sync.dma_start(
                            dbg_d.ap()[:, 0, 0:tq],
                            qt_sb[1][:].bitcast(F32))
                        nc.sync.dma_start(
                            dbg_d.ap()[:, 1, :], kt_sb[1][:].bitcast(F32))
                    # tail: each chunk's three q-subtiles share one PSUM
                    # tile (independent accumulation regions) so a single
                    # eviction and two out-DMAs (HWDGE generation is the
                    # 700ns/DMA tail bottleneck) drain the last head.
                    h3 = HPC - 1
                    with tc.tile_pool(name="avp", bufs=3,
                                      space="PSUM") as avp_pool:
                        for j in range(nch):
                            avp = avp_pool.tile([128, ns, DH + 1], F32,
                                                tag="o", name="av3c")
                            for si, (o, w) in enumerate(subs):
                                for t in range(nkt):
                                    nc.tensor.matmul(
                                        avp[0:w, si, :],
                                        e_tiles[(h3, t)][:, j, o:o + w],
                                        v_sb[:, t, h3, :],
                                        start=(t == 0), stop=(t == nkt - 1),
                                        skip_group_check=True,
                                    )
                            if j % 2 == 0:
                                nc.scalar.copy(y_sb[:, j, :, h3, :], avp[:])
                            else:
                                nc.vector.tensor_copy(
                                    y_sb[:, j, :, h3, :], avp[:])
                            if j == nch - 2:
                                nc.sync.dma_start(
                                    out_d.ap()[:, 0:j + 1, :, h3, :],
                                    y_sb[:, 0:j + 1, :, h3, :])
                            elif j == nch - 1:
                                nc.sync.dma_start(
                                    out_d.ap()[:, j:j + 1, :, h3, :],
                                    y_sb[:, j:j + 1, :, h3, :])

    nc.compile()
    return nc


def _get_nc(tp, nkt, cw, nch, tq, with_bias):
    key = (tp, nkt, cw, nch, tq, with_bias)
    if key not in _CACHE:
        _CACHE[key] = _build(tp, nkt, cw, nch, tq, with_bias)
    return _CACHE[key]


def kernel(x, Wq, bq, Wk, bk, Wv, bv, mask):
    x = np.asarray(x, dtype=np.float32)
    Wq = np.asarray(Wq, dtype=np.float32)
    bq = np.asarray(bq, dtype=np.float32)
    Wk = np.asarray(Wk, dtype=np.float32)
    bk = np.asarray(bk, dtype=np.float32)
    Wv = np.asarray(Wv, dtype=np.float32)
    bv = np.asarray(bv, dtype=np.float32)
    mask = np.asarray(mask)

    idxs = [np.nonzero(mask[b] != 0)[0] for b in range(B)]
    tvs = [len(ix) for ix in idxs]
    tp, nkt, cw, nch, tq = _pick_dims(max(max(tvs), 1))
    with_bias = bool(np.any(bq) or np.any(bk) or np.any(bv))
    nc = _get_nc(tp, nkt, cw, nch, tq, with_bias)
    subs = _subtiles(cw)

    onesv = np.ones((128, nkt * HPC), NPBF)

    # per-batch tensors: fp8 residual-split x planes
    xhs, xls, ebs = [], [], []
    for b in range(B):
        xt = np.zeros((C, tp), np.float32)
        if tvs[b]:
            xt[:, :tvs[b]] = x[b][idxs[b]].T
        xh = xt.astype(NP8H)
        xl = (xt - xh.astype(np.float32)).astype(NP8L)
        xhs.append(xh)
        xls.append(xl)
        eb = np.full(tp, -1e30, np.float32)
        eb[:tvs[b]] = 0.0
        ebs.append(eb.reshape(nkt, 128).T.copy())

    in_maps = []
    for core in range(N_CORES):
        b, hg = core // HPC, core % HPC
        cs = hg * CSL
        if with_bias:
            bqs, bks = bq * WS, bk * WS
            misc = np.concatenate([
                ebs[b],
                np.stack([bqs[cs:cs + 128], bqs[cs + 128:cs + 256],
                          bks[cs:cs + 128], bks[cs + 128:cs + 256]],
                         axis=1),
            ], axis=1)
        else:
            misc = ebs[b]
        def planes(W):
            w = W[:, cs:cs + CSL].astype(np.float32) * WS
            wh = w.astype(NP8H)
            wl = (w - wh.astype(np.float32)).astype(NP8L)
            return wh, wl

        def swz2(w):
            # [C, CSL] -> [2 d-half, 128 partition, NCT c-tile, 128]
            return np.ascontiguousarray(
                w.reshape(NCT, 128, 2, 128).transpose(2, 1, 0, 3))

        def swz(w):
            return np.ascontiguousarray(
                w.reshape(NCT, 128, CSL).transpose(1, 0, 2))

        wqh, wql = planes(Wq)
        wkh, wkl = planes(Wk)
        wvh, wvl = planes(Wv)
        # pack [2 d-half][4 plane][128][NCT][128] then split halves
        wqk = np.stack([swz2(wqh).view(np.uint8),
                        swz2(wql).view(np.uint8),
                        swz2(wkh).view(np.uint8),
                        swz2(wkl).view(np.uint8)], axis=2)
        wvp = np.stack([swz(wvh).view(np.uint8),
                        swz(wvl).view(np.uint8)], axis=1)
        im = {
            "xh": xhs[b],
            "xl": xls[b],
            "onesv": onesv,
            "w0": np.ascontiguousarray(wqk[0]),
            "w1": np.ascontiguousarray(wqk[1]),
            "wv": np.ascontiguousarray(wvp),
            "misc": np.ascontiguousarray(misc),
        }
        if with_bias:
            im["bv"] = np.ascontiguousarray(
                (bv[cs:cs + CSL] * WS).reshape(1, -1))
        in_maps.append(im)

    try:
        res = bass_utils.run_bass_kernel_spmd(
            nc, in_maps, core_ids=list(range(N_CORES)), trace=False)
    except Exception:
        # transient axon-worker/NRT failures recover on retry
        res = bass_utils.run_bass_kernel_spmd(
            nc, in_maps, core_ids=list(range(N_CORES)), trace=False)

    y = np.zeros((B, T, C), np.float32)
    for core in range(N_CORES):
        b, hg = core // HPC, core % HPC
        out = res.results[core]["out"]      # [128, nch, ns, HPC, DH+1]
        ix, tv = idxs[b], tvs[b]
        if not tv:
            continue
        split3 = False
        out3 = res.results[core]["out3"]
        for h in range(HPC):
            col = hg * CSL + h * DH
            for j in range(nch):
                if split3 and h == HPC - 1:
                    q0 = j * cw
                    n = min(cw, tv - q0)
                    if n <= 0:
                        continue
                    blk = out3[:, j, 0:n]                # [65, n]
                    y[b, ix[q0:q0 + n], col:col + DH] = (
                        blk[:DH] / blk[DH:DH + 1] / WS).T
                    continue
                for si, (o, w) in enumerate(subs):
                    q0 = j * cw + o
                    n = min(w, tv - q0)
                    if n <= 0:
                        continue
                    blk = out[0:n, j, si, h, :].astype(np.float32)
                    numer = blk[:, :DH]
                    denom = blk[:, DH:DH + 1]
                    y[b, ix[q0:q0 + n], col:col + DH] = (
                        numer / denom / WS)
    return y


# revision 68
# speedup vs baseline: 1.0026x; 1.0026x over previous
"""Bass/Trainium2 kernel for masked (padding) multi-head self-attention.

Problem: B=2, T=2048, C=1024, H=16 heads of DH=64.
  q/k/v = x @ W* + b*  ->  att = softmax(mask(q k^T / 8))  ->  y = att @ v

Sharding over 8 NeuronCores: core = (batch b, head-group hg) with
b = core // 4, hg = core % 4; each core computes 4 heads for one batch
element (its [T, 256] slice of q/k/v from the Wq/Wk/Wv column slice).

Host-side preprocessing (inside kernel()):
  - Only valid (mask==1) tokens are gathered; the k-dim is padded to tp
    (multiple of 128 for PE k-tiles), the q/free dim trimmed to
    tq = nch*cw >= max valid (cw a multiple of 8: fp32r matmuls reject
    odd free sizes, s3d3_mm_fp32r_restrictions).
  - x ships as fp8 residual-split planes xh=e4m3(x), xl=e5m2(x-xh);
    each W ships as hi/lo planes of 16*W (power-of-2 prescale keeps the
    hi plane in e4m3 normal range), pre-swizzled to [d-half, partition,
    c-tile, 128] and packed 4-planes-per-uint8-container so each
    critical DMA is one contiguous transfer per partition row.

Device compute (per core), dtype/layout choices from an error study
(split-fp8 proj + bf16 e/v/out ~ 5e-3 metric vs the 2e-2 gate):
  qT/kT/v: 3-term DoubleRow fp8 matmuls (xh*Wh + xl*Wh + xh*Wl) over
    c-tile pairs at 0.5 cycles/row -- 25% cheaper than bf16 with ~2x
    better accuracy (effective ~12-bit mantissa).  qT/kT evict to f32r.
  sT[k,q] = sum_d kT[d,k] qT[d,q]   (f32r x f32r, 1.0 c/row at cw>=256)
  e = exp(s_raw/(8*256) + ebias_t)  (ACT; bias column kills pad k-rows;
    e stored bf16)
  y[q,dd] = sum_k e[k,q] vaug[k,dd] (lhsT=e stationary, rhs=v bf16
    moving, out [q-subtile, 65] accumulated over all k in PSUM; column
    64 of vaug is ones -> softmax denominator).  y staged bf16.
Normalization (numer/denom/16) and scatter back to [T, C] on host.

Schedule (TimelineSim 58083 ns/core vs 77354 baseline; HW-verified
rel err 5.1e-3):
  head ~13.4us: DMA-roofline on Wq/Wk-d0 + x planes (2.9MB); the
    d-tile-0 q projection + k chunk-0 chase the per-ct-pair transfers
    ct-major with 6 open PSUM groups; evictions alternate ACT/DVE.
  stream ~39.7us: ACT-bound, 36 exps near-dense.  Heads 0/1 sweep as a
    pair (PE-heavy phase: remaining k-d0/d1/v units drip in as budgeted
    fillers); heads 2 and 3 sweep singly so e(2,*) completes early and
    av(2) streams during head-3's exps.  AV runs in flipped orientation
    (out [q,65], 65-cycle instructions, no SBUF accumulator chain);
    per-chunk out DMAs overlap the stream.
  tail ~5.0us: head-3's final score tile runs through the ops pool as 3
    per-chunk exps (frees all sps banks one slot early); its AV chunks
    share one PSUM tile per chunk (independent accumulation regions,
    skip_group_check) for single evictions and only two out-DMAs
    (HWDGE generation, ~700ns/DMA, is the tail bottleneck).
Known-negative experiments (reverted): chunk-oriented av3 A/B k-splits
(ops-slot congestion); per-sub tail DMAs and ACT-queue out-DMAs (DGE
overhead/exp-slot theft); eager fillers beyond ~1.3us/slot budgets.
"""

import math
import sys

sys.path.insert(0, "/opt/trn_rl_repo")

import ml_dtypes
import numpy as np

import concourse.bacc as bacc
import concourse.mybir as mybir
import concourse.tile as tile
from concourse import bass_utils

F32 = mybir.dt.float32
F32R = mybir.dt.float32r
BF16 = mybir.dt.bfloat16
F8H = mybir.dt.float8e4
F8L = mybir.dt.float8e5
DR = mybir.MatmulPerfMode.DoubleRow
AF = mybir.ActivationFunctionType
NPBF = ml_dtypes.bfloat16
NP8H = ml_dtypes.float8_e4m3
NP8L = ml_dtypes.float8_e5m2
WS = 16.0  # power-of-2 prescale keeping fp8 W planes in normal range

B, T, C, H = 2, 2048, 1024, 16
DH = C // H            # 64
HPC = 4                # heads per core
CSL = HPC * DH         # 256, per-core column slice of C
N_CORES = 8
NCT = C // 128         # 8 contraction tiles over C

_CACHE: dict = {}


def _pick_dims(max_valid: int):
    """k-dim tiles (nkt, tp) and q-dim chunks (nch, cw, tq)."""
    mt = max(max_valid, 1)
    nkt = max(2, math.ceil(mt / 128))
    tp = nkt * 128
    nch = max(1, math.ceil(mt / 512))
    if nch < 3 and nch * 512 < tp:
        nch = min(3, math.ceil(tp / 512))
    # fp32r matmuls reject odd free sizes (s3d3_mm_fp32r_restrictions):
    # keep chunk widths a multiple of 8
    cw = min(512, math.ceil(mt / nch / 8) * 8)
    while nch * cw < mt:
        cw = min(512, cw + 8)
        if nch * cw < mt and cw == 512:
            nch += 1
    tq = nch * cw
    return tp, nkt, cw, nch, tq


def _subtiles(cw: int):
    offs, widths = [], []
    o = 0
    while o < cw:
        w = min(128, cw - o)
        offs.append(o)
        widths.append(w)
        o += w
    return list(zip(offs, widths))


def _build(tp, nkt, cw, nch, tq, with_bias):
    nc = bacc.Bacc("TRN2", target_bir_lowering=False, debug=False,
                   num_devices=N_CORES)

    # x and W ship as fp8 residual-split planes (hi=e4m3, lo=e5m2);
    # projections run as 3-term DoubleRow matmuls (hi*hi + lo*hi + hi*lo)
    # at 0.5 cycles/row -- 25% cheaper than bf16 with ~2x less error.
    # W planes are pre-swizzled [d-half, partition, c-tile, 128] so every
    # half-DMA is contiguous per partition row (no 256B-piece penalty).
    xh_d = nc.dram_tensor("xh", [C, tp], F8H, kind="ExternalInput")
    xl_d = nc.dram_tensor("xl", [C, tp], F8L, kind="ExternalInput")
    # qh/ql/kh/kl planes packed per d-half into one uint8 container so
    # each is a single contiguous DMA; slices are bitcast at use sites
    w0_d = nc.dram_tensor("w0", [128, 4, NCT, 128], mybir.dt.uint8,
                          kind="ExternalInput")
    w1_d = nc.dram_tensor("w1", [128, 4, NCT, 128], mybir.dt.uint8,
                          kind="ExternalInput")
    wv_d = nc.dram_tensor("wv", [128, 2, NCT, CSL], mybir.dt.uint8,
                          kind="ExternalInput")
    # misc: col 0..nkt-1 = ebias per k-tile; col nkt..nkt+3 = bq/bk halves
    nmc = nkt + (4 if with_bias else 0)
    misc_d = nc.dram_tensor("misc", [128, nmc], F32, kind="ExternalInput")
    onesv_d = nc.dram_tensor("onesv", [128, nkt * HPC], BF16,
                             kind="ExternalInput")
    if with_bias:
        bv_d = nc.dram_tensor("bv", [1, CSL], F32, kind="ExternalInput")
    subs = _subtiles(cw)
    ns = len(subs)
    out_d = nc.dram_tensor("out", [128, nch, ns, HPC, DH + 1], BF16,
                           kind="ExternalOutput")
    # head-3 leaves in [dd, chunk] orientation (tail-optimized path)
    out3_d = nc.dram_tensor("out3", [DH + 1, nch, cw], F32,
                            kind="ExternalOutput")
    import os
    _dbg = bool(os.environ.get("KERNEL_DEBUG"))
    if _dbg:
        dbg_d = nc.dram_tensor("dbg", [128, 2, tp], F32,
                               kind="ExternalOutput")

    chunks = [(j * cw, cw) for j in range(nch)]
    seq_heads = nkt >= 12          # SBUF can't hold 4 heads of e-tiles

    with tile.TileContext(nc) as tc:
        with tc.tile_pool(name="const", bufs=1) as cp:
            xh_sb = cp.tile([128, NCT, tp], F8H, tag="xh")
            xl_sb = cp.tile([128, NCT, tp], F8L, tag="xl")
            w01_sb = [cp.tile([128, 4, NCT, 128], mybir.dt.uint8,
                              tag=f"w{p}", name=f"w{p}") for p in range(2)]
            wv_sb = cp.tile([128, 2, NCT, CSL], mybir.dt.uint8, tag="wv")
            misc_sb = cp.tile([128, nmc], F32, tag="misc")
            qt_sb = [cp.tile([128, tq], F32R, tag=f"qt{p}", name=f"qt{p}")
                     for p in range(2)]
            kt_sb = [cp.tile([128, tp], F32R, tag=f"kt{p}", name=f"kt{p}")
                     for p in range(2)]
            v_sb = cp.tile([128, nkt, HPC, DH + 1], BF16, tag="v")
            y_sb = cp.tile([128, nch, ns, HPC, DH + 1], BF16, tag="y")
            y3_sb = cp.tile([DH + 1, nch, cw], F32, tag="y3")
            ebias_sb = misc_sb[:, 0:nkt]
            if with_bias:
                bqk_sb = misc_sb[:, nkt:nkt + 4]
                bv_sb = cp.tile([1, CSL], F32R, tag="bv")
                ones_sb = cp.tile([1, 128], F32R, tag="ones")

            scratch = cp.tile([1, 8], F32, tag="scratch")

            xh_r = xh_d.ap().rearrange("(i p) t -> p i t", p=128)
            xl_r = xl_d.ap().rearrange("(i p) t -> p i t", p=128)
            # critical-path DMAs in strict SP-queue order: Wq/Wk d0 plane
            # halves, the x hi/lo streams (d-tile-0 projection chases them
            # per ct-pair), then d1 halves and Wv off the critical path.
            nc.sync.dma_start(w01_sb[0][:], w0_d.ap()[:])
            nc.sync.dma_start(misc_sb[:], misc_d.ap()[:])
            for i in range(0, NCT, 2):
                nc.sync.dma_start(xh_sb[:, i:i + 2, :], xh_r[:, i:i + 2, :])
                nc.sync.dma_start(xl_sb[:, i:i + 2, :], xl_r[:, i:i + 2, :])
            nc.sync.dma_start(w01_sb[1][:], w1_d.ap()[:])
            nc.sync.dma_start(wv_sb[:], wv_d.ap()[:])
            if with_bias:
                nc.sync.dma_start(bv_sb[:], bv_d.ap()[:].bitcast(F32R))
                nc.gpsimd.memset(ones_sb[:], 1.0)

            # denominator ones-column of vaug; zero the kT columns beyond
            # the projected range (pad k-tokens; killed by ebias anyway but
            # must be finite)
            nc.sync.dma_start(
                v_sb[:, :, :, DH],
                onesv_d.ap().rearrange("p (t h) -> p t h", h=HPC))
            if tq < tp:
                nc.gpsimd.memset(kt_sb[0][:, tq:tp].bitcast(F32), 0.0)
                nc.gpsimd.memset(kt_sb[1][:, tq:tp].bitcast(F32), 0.0)

            # warm the ACT exp table during the DMA window
            nc.gpsimd.memset(scratch[:], 0.0)
            nc.scalar.activation(scratch[:], scratch[:], AF.Exp)

            def evict_qk(o_ap, ps_ap, bcol, alt=1):
                # PSUM reads: DVE/ACT only (GPSIMD cannot access PSUM);
                # alternating engines halves the eviction chain on the
                # critical path out of phase A.
                if with_bias:
                    if alt % 2 == 0:
                        nc.scalar.activation(o_ap, ps_ap, AF.Identity,
                                             bias=bqk_sb[:, bcol:bcol + 1])
                    else:
                        nc.vector.tensor_scalar_add(o_ap, ps_ap,
                                                    bqk_sb[:, bcol:bcol + 1])
                else:
                    if alt % 2 == 0:
                        nc.scalar.copy(o_ap, ps_ap)
                    else:
                        nc.vector.tensor_copy(o_ap, ps_ap)

            NPAIR = NCT // 2
            QK_TERMS = (0, 1)  # matrix index: 0 = q, 1 = k

            def qk_terms(mi, p, cts):
                wt = w01_sb[p]
                wh = wt[:, 2 * mi, cts, :].bitcast(F8H)
                wl = wt[:, 2 * mi + 1, cts, :].bitcast(F8L)
                return ((wh, xh_sb), (wh, xl_sb), (wl, xh_sb))

            def proj_chunks(pool, tag, p, work):
                # ct-pair-major emission with the accumulation groups open
                # so the DoubleRow matmuls chase the x-plane DMAs; work
                # items are (w_pair, o_sb, bias-col-base, chunk-off, w).
                tiles = [pool.tile([128, cw], F32, tag=tag, name="pqk")
                         for _ in work]
                for cp_i in range(NPAIR):
                    cts = slice(2 * cp_i, 2 * cp_i + 2)
                    for ps, (w_pair, o_sb, bc, off, w) in zip(tiles, work):
                        for ti, (lhs, x_sb) in enumerate(
                                qk_terms(w_pair, p, cts)):
                            nc.tensor.matmul(
                                ps[:, 0:w],
                                lhs,
                                x_sb[:, cts, off:off + w],
                                start=(cp_i == 0 and ti == 0),
                                stop=(cp_i == NPAIR - 1 and ti == 2),
                                perf_mode=DR,
                            )
                for n, (ps, (w_pair, o_sb, bc, off, w)) in enumerate(
                        zip(tiles, work)):
                    evict_qk(o_sb[p][:, off:off + w], ps[:, 0:w], bc + p, n)


            # phase A: qkT d-tile-0 projection with 6 psum slots so all six
            # accumulation groups pipeline with the incoming xt DMAs.
            with tc.tile_pool(name="pa", bufs=6, space="PSUM") as pa:
                # warm the PE (HAM clock gate) during the DMA window
                wsc = cp.tile([128, 16], F32, tag="wsc")
                nc.gpsimd.memset(wsc[:], 0.0)
                for _ in range(60):
                    wps = pa.tile([16, 16], F32, tag="a", name="wps")
                    nc.tensor.matmul(wps[:], wsc[:, 0:16], wsc[:],
                                     start=True, stop=True)
                # q d0 all chunks + k d0 chunk 0 only: 4 matmuls per ct
                # keeps the chase under the per-tile DMA time; k d0 ch1/2
                # run as early main-loop units (first needed at t=3).
                proj_chunks(pa, "a", 0,
                            [(QK_TERMS[0], qt_sb, 0, off, w)
                             for off, w in chunks]
                            + [(QK_TERMS[1], kt_sb, 2, chunks[0][0],
                                chunks[0][1])])

            ebufs = (nkt + 3) if seq_heads else (4 * nkt + 2)

            with (
                tc.tile_pool(name="ops", bufs=2, space="PSUM") as ops,
                tc.tile_pool(name="epool", bufs=ebufs) as ep,
            ):
                e_tiles: dict = {}
                chunk_cnt: dict = {}

                def note_evict(h, j, si=None):
                    c = chunk_cnt.get((h, j), 0) + 1
                    chunk_cnt[(h, j)] = c
                    if c == ns:
                        nc.sync.dma_start(out_d.ap()[:, j, :, h, :],
                                          y_sb[:, j, :, h, :])

                def proj_v_unit(t):
                    ps = ops.tile([128, CSL], F32, tag="o", name="pv")
                    tsl = slice(t * 128, (t + 1) * 128)
                    for cp_i in range(NPAIR):
                        cts = slice(2 * cp_i, 2 * cp_i + 2)
                        wvh = wv_sb[:, 0, cts, :].bitcast(F8H)
                        wvl = wv_sb[:, 1, cts, :].bitcast(F8L)
                        terms = ((xh_sb[:, cts, tsl], wvh),
                                 (xl_sb[:, cts, tsl], wvh),
                                 (xh_sb[:, cts, tsl], wvl))
                        for ti, (xs, wvs) in enumerate(terms):
                            nc.tensor.matmul(
                                ps[:],
                                xs,
                                wvs,
                                start=(cp_i == 0 and ti == 0),
                                stop=(not with_bias
                                      and cp_i == NPAIR - 1 and ti == 2),
                                perf_mode=DR,
                            )
                    if with_bias:
                        nc.tensor.matmul(ps[:], ones_sb[:], bv_sb[:],
                                         start=False, stop=True)
                    nc.vector.tensor_copy(
                        v_sb[:, t, :, 0:DH],
                        ps[:].rearrange("p (h d) -> p h d", h=HPC),
                    )

                def qkd1_unit(w_pair, o_sb, bc, off, w, n):
                    ps = ops.tile([128, cw], F32, tag="o", name="pqk1")
                    for cp_i in range(NPAIR):
                        cts = slice(2 * cp_i, 2 * cp_i + 2)
                        for ti, (lhs, x_sb) in enumerate(
                                qk_terms(w_pair, 1, cts)):
                            nc.tensor.matmul(
                                ps[:, 0:w],
                                lhs,
                                x_sb[:, cts, off:off + w],
                                start=(cp_i == 0 and ti == 0),
                                stop=(cp_i == NPAIR - 1 and ti == 2),
                                perf_mode=DR,
                            )
                    evict_qk(o_sb[1][:, off:off + w], ps[:, 0:w], bc + 1)

                def scores(sps_pool, h, t, filler=None, split_exp=False):
                    pd, po = h // 2, (h % 2) * 64
                    qt_h, kt_h = qt_sb[pd], kt_sb[pd]
                    ps = sps_pool.tile([128, nch, 512], F32, tag="s",
                                       name="sps")
                    for j, (off, w) in enumerate(chunks):
                        nc.tensor.matmul(
                            ps[:, j, 0:w],
                            kt_h[po:po + 64, t * 128:(t + 1) * 128],
                            qt_h[po:po + 64, off:off + w],
                            start=True, stop=True,
                        )
                    if filler:
                        filler(t)
                    e_t = ep.tile([128, nch, cw], BF16, tag="e", name="e")
                    if split_exp:
                        # per-chunk exps let the final AV/evict/DMA chain
                        # pipeline chunk-by-chunk behind the last exp
                        for j in range(nch):
                            nc.scalar.activation(
                                e_t[:, j, :], ps[:, j, 0:cw], AF.Exp,
                                bias=ebias_sb[:, t:t + 1],
                                scale=0.125 / (WS * WS),
                            )
                    else:
                        nc.scalar.activation(
                            e_t[:], ps[:, :, 0:cw], AF.Exp,
                            bias=ebias_sb[:, t:t + 1],
                            scale=0.125 / (WS * WS),
                        )
                    e_tiles[(h, t)] = e_t

                def scores_pair(sps_pool, hA, hB, t, filler=None):
                    # hA/hB share a qT/kT d-tile at partition offsets 0/64;
                    # alternating the chunk matmuls lets the PE row-groups
                    # overlap the two heads' streams.
                    pd = hA // 2
                    qt_h, kt_h = qt_sb[pd], kt_sb[pd]
                    pss = {}
                    for h in (hA, hB):
                        pss[h] = sps_pool.tile([128, nch, 512], F32, tag="s",
                                               name="sps")
                    for j, (off, w) in enumerate(chunks):
                        for h in (hA, hB):
                            po = (h % 2) * 64
                            nc.tensor.matmul(
                                pss[h][:, j, 0:w],
                                kt_h[po:po + 64, t * 128:(t + 1) * 128],
                                qt_h[po:po + 64, off:off + w],
                                start=True, stop=True,
                            )
                    if filler:
                        filler(t)
                    for h in (hA, hB):
                        e_t = ep.tile([128, nch, cw], BF16, tag="e", name="e")
                        nc.scalar.activation(
                            e_t[:], pss[h][:, :, 0:cw], AF.Exp,
                            bias=ebias_sb[:, t:t + 1],
                            scale=0.125 / (WS * WS),
                        )
                        e_tiles[(h, t)] = e_t

                def scores_last(h, t):
                    # final tile of the last head: per-chunk psums from the
                    # ops pool + per-chunk exps.  The sps banks are all
                    # free one slot earlier, so the tail AV groups
                    # pre-accumulate, and AV/evict/DMA pipeline per chunk
                    # behind the three chunk-exps.
                    pd, po = h // 2, (h % 2) * 64
                    qt_h, kt_h = qt_sb[pd], kt_sb[pd]
                    e_t = ep.tile([128, nch, cw], BF16, tag="e", name="e")
                    for j, (off, w) in enumerate(chunks):
                        ps = ops.tile([128, cw], F32, tag="o", name="sl")
                        nc.tensor.matmul(
                            ps[:, 0:w],
                            kt_h[po:po + 64, t * 128:(t + 1) * 128],
                            qt_h[po:po + 64, off:off + w],
                            start=True, stop=True,
                        )
                        nc.scalar.activation(
                            e_t[:, j, :], ps[:, 0:cw], AF.Exp,
                            bias=ebias_sb[:, t:t + 1],
                            scale=0.125 / (WS * WS),
                        )
                    e_tiles[(h, t)] = e_t

                def av_sub(pool, h, j, s_off, s_w, si, act_evict=False,
                           ts=None, accum=False, note=True):
                    if ts is None:
                        ts = range(nkt)
                    avp = pool.tile([128, DH + 1], F32, tag="o", name="av")
                    for i, t in enumerate(ts):
                        nc.tensor.matmul(
                            avp[0:s_w, :],
                            e_tiles[(h, t)][:, j, s_off:s_off + s_w],
                            v_sb[:, t, h, :],
                            start=(i == 0), stop=(i == len(ts) - 1),
                        )
                    if accum:
                        nc.vector.tensor_add(
                            y_sb[0:s_w, j, si, h, :],
                            y_sb[0:s_w, j, si, h, :], avp[0:s_w, :])
                    elif act_evict:
                        nc.scalar.copy(y_sb[0:s_w, j, si, h, :], avp[0:s_w, :])
                    else:
                        nc.vector.tensor_copy(
                            y_sb[0:s_w, j, si, h, :], avp[0:s_w, :])
                    if note:
                        note_evict(h, j, si)

                if seq_heads:
                    with tc.tile_pool(name="sps", bufs=2,
                                      space="PSUM") as sps_pool:
                        for off, w in chunks[1:]:
                            proj_chunks(ops, "o", 0,
                                        [(QK_TERMS[1], kt_sb, 2, off, w)])
                        proj_chunks(ops, "o", 1,
                                    [(QK_TERMS[0], qt_sb, 0, off, w)
                                     for off, w in chunks]
                                    + [(QK_TERMS[1], kt_sb, 2, off, w)
                                       for off, w in chunks])
                        for t in range(nkt):
                            proj_v_unit(t)
                        for h in range(HPC):
                            for t in range(nkt):
                                scores(sps_pool, h, t)
                            for j in range(nch):
                                for si, (o, w) in enumerate(subs):
                                    av_sub(ops, h, j, o, w, si)
                else:
                    # fillers for the pair(0,1) sweep: v tiles + the
                    # d-tile-1 q/k projection, one unit per exp-slot; the
                    # overflow drains into the later single-head sweeps
                    # where the PE is otherwise starved.
                    units = [("k0", (off, w)) for off, w in chunks[1:]]
                    units += [("v", t) for t in range(nkt)]
                    n = 0
                    for w_pair, o_sb, bc in ((QK_TERMS[0], qt_sb, 0),
                                             (QK_TERMS[1], kt_sb, 2)):
                        for off, w in chunks:
                            units.insert(len(chunks) - 1 + 2 * n + 1,
                                         ("d1", (w_pair, o_sb, bc, off, w,
                                                 n)))
                            n += 1

                    def emit_unit(units):
                        if not units:
                            return False
                        kind, a = units.pop(0)
                        if kind == "v":
                            proj_v_unit(a)
                        elif kind == "k0":
                            proj_chunks(ops, "o", 0,
                                        [(QK_TERMS[1], kt_sb, 2, a[0],
                                          a[1])])
                        else:
                            qkd1_unit(*a)
                        return True

                    def subwork(h):
                        return [(h, j, si, o, w) for j in range(nch)
                                for si, (o, w) in enumerate(subs)]

                    av01 = subwork(0) + subwork(1)
                    av2 = subwork(2)
                    av3 = subwork(3)
                    split3 = False
                    ka3 = list(range(nkt - 3))
                    kb3 = list(range(nkt - 3, nkt))

                    def av3_chunk(pool, j, ts, accum):
                        # old-orientation AV for the tail head: out
                        # [dd, chunk] costs more PE but only nch groups,
                        # each finishing 144ns after its last e-tile.
                        p3 = pool.tile([DH + 1, cw], F32, tag="o", name="av3")
                        for i, t in enumerate(ts):
                            nc.tensor.matmul(
                                p3[:],
                                v_sb[:, t, HPC - 1, :],
                                e_tiles[(HPC - 1, t)][:, j, :],
                                start=(i == 0), stop=(i == len(ts) - 1),
                            )
                        if accum:
                            nc.vector.tensor_add(y3_sb[:, j, :],
                                                 y3_sb[:, j, :], p3[:])
                            nc.sync.dma_start(out3_d.ap()[:, j, :],
                                              y3_sb[:, j, :])
                        else:
                            nc.vector.tensor_copy(y3_sb[:, j, :], p3[:])

                    with tc.tile_pool(name="sps", bufs=2,
                                      space="PSUM") as sps_pool:
                        def filler01(t):
                            if t is not None and t < 1:
                                return
                            budget = 1300
                            while budget > 0 and units:
                                emit_unit(units)
                                budget -= 800

                        # heads 0/1 paired (PE-heavy phase), then heads 2
                        # and 3 swept singly: e(2,*) completes a full sweep
                        # early, so av(2) streams during head-3's exps and
                        # only av(3) remains after the last exp.
                        for t in range(nkt):
                            scores_pair(sps_pool, 0, 1, t, filler=filler01)

                        def filler2(t):
                            budget = 500
                            while budget > 0:
                                if units:
                                    emit_unit(units)
                                    budget -= 800
                                elif av01:
                                    h, j, si, o, w = av01.pop(0)
                                    av_sub(ops, h, j, o, w, si)
                                    budget -= 260
                                else:
                                    return

                        for t in range(nkt):
                            scores(sps_pool, 2, t, filler=filler2)
                        while units:
                            emit_unit(units)

                        def filler3(t):
                            budget = 500
                            while budget > 0:
                                if units:
                                    emit_unit(units)
                                    budget -= 800
                                elif av01:
                                    h, j, si, o, w = av01.pop(0)
                                    av_sub(ops, h, j, o, w, si)
                                    budget -= 260
                                elif av2:
                                    h, j, si, o, w = av2.pop(0)
                                    av_sub(ops, h, j, o, w, si)
                                    budget -= 260
                                else:
                                    return

                        for t in range(nkt - 1):
                            scores(sps_pool, 3, t, filler=filler3)
                        scores_last(3, nkt - 1)
                        while av01:
                            h, j, si, o, w = av01.pop(0)
                            av_sub(ops, h, j, o, w, si)
                        while av2:
                            h, j, si, o, w = av2.pop(0)
                            av_sub(ops, h, j, o, w, si)

                    if _dbg:
                        nc.sync.dma_start(
                            dbg_d.ap()[:, 0, 0:tq],
                            qt_sb[1][:].bitcast(F32))
                        nc.sync.dma_start(
                            dbg_d.ap()[:, 1, :], kt_sb[1][:].bitcast(F32))
                    # tail: each chunk's three q-subtiles share one PSUM
                    # tile (independent accumulation regions) so a single
                    # eviction and two out-DMAs (HWDGE generation is the
                    # 700ns/DMA tail bottleneck) drain the last head.
                    h3 = HPC - 1
                    with tc.tile_pool(name="avp", bufs=3,
                                      space="PSUM") as avp_pool:
                        for j in range(nch):
                            avp = avp_pool.tile([128, ns, DH + 1], F32,
                                                tag="o", name="av3c")
                            for si, (o, w) in enumerate(subs):
                                for t in range(nkt):
                                    nc.tensor.matmul(
                                        avp[0:w, si, :],
                                        e_tiles[(h3, t)][:, j, o:o + w],
                                        v_sb[:, t, h3, :],
                                        start=(t == 0), stop=(t == nkt - 1),
                                        skip_group_check=True,
                                    )
                            nc.vector.tensor_copy(
                                y_sb[:, j, :, h3, :], avp[:])
                            if j == nch - 2:
                                nc.sync.dma_start(
                                    out_d.ap()[:, 0:j + 1, :, h3, :],
                                    y_sb[:, 0:j + 1, :, h3, :])
                            elif j == nch - 1:
                                nc.sync.dma_start(
                                    out_d.ap()[:, j:j + 1, :, h3, :],
                                    y_sb[:, j:j + 1, :, h3, :])

    nc.compile()
    return nc


def _get_nc(tp, nkt, cw, nch, tq, with_bias):
    key = (tp, nkt, cw, nch, tq, with_bias)
    if key not in _CACHE:
        _CACHE[key] = _build(tp, nkt, cw, nch, tq, with_bias)
    return _CACHE[key]


def kernel(x, Wq, bq, Wk, bk, Wv, bv, mask):
    x = np.asarray(x, dtype=np.float32)
    Wq = np.asarray(Wq, dtype=np.float32)
    bq = np.asarray(bq, dtype=np.float32)
    Wk = np.asarray(Wk, dtype=np.float32)
    bk = np.asarray(bk, dtype=np.float32)
    Wv = np.asarray(Wv, dtype=np.float32)
    bv = np.asarray(bv, dtype=np.float32)
    mask = np.asarray(mask)

    idxs = [np.nonzero(mask[b] != 0)[0] for b in range(B)]
    tvs = [len(ix) for ix in idxs]
    tp, nkt, cw, nch, tq = _pick_dims(max(max(tvs), 1))
    with_bias = bool(np.any(bq) or np.any(bk) or np.any(bv))
    nc = _get_nc(tp, nkt, cw, nch, tq, with_bias)
    subs = _subtiles(cw)

    onesv = np.ones((128, nkt * HPC), NPBF)

    # per-batch tensors: fp8 residual-split x planes
    xhs, xls, ebs = [], [], []
    for b in range(B):
        xt = np.zeros((C, tp), np.float32)
        if tvs[b]:
            xt[:, :tvs[b]] = x[b][idxs[b]].T
        xh = xt.astype(NP8H)
        xl = (xt - xh.astype(np.float32)).astype(NP8L)
        xhs.append(xh)
        xls.append(xl)
        eb = np.full(tp, -1e30, np.float32)
        eb[:tvs[b]] = 0.0
        ebs.append(eb.reshape(nkt, 128).T.copy())

    in_maps = []
    for core in range(N_CORES):
        b, hg = core // HPC, core % HPC
        cs = hg * CSL
        if with_bias:
            bqs, bks = bq * WS, bk * WS
            misc = np.concatenate([
                ebs[b],
                np.stack([bqs[cs:cs + 128], bqs[cs + 128:cs + 256],
                          bks[cs:cs + 128], bks[cs + 128:cs + 256]],
                         axis=1),
            ], axis=1)
        else:
            misc = ebs[b]
        def planes(W):
            w = W[:, cs:cs + CSL].astype(np.float32) * WS
            wh = w.astype(NP8H)
            wl = (w - wh.astype(np.float32)).astype(NP8L)
            return wh, wl

        def swz2(w):
            # [C, CSL] -> [2 d-half, 128 partition, NCT c-tile, 128]
            return np.ascontiguousarray(
                w.reshape(NCT, 128, 2, 128).transpose(2, 1, 0, 3))

        def swz(w):
            return np.ascontiguousarray(
                w.reshape(NCT, 128, CSL).transpose(1, 0, 2))

        wqh, wql = planes(Wq)
        wkh, wkl = planes(Wk)
        wvh, wvl = planes(Wv)
        # pack [2 d-half][4 plane][128][NCT][128] then split halves
        wqk = np.stack([swz2(wqh).view(np.uint8),
                        swz2(wql).view(np.uint8),
                        swz2(wkh).view(np.uint8),
                        swz2(wkl).view(np.uint8)], axis=2)
        wvp = np.stack([swz(wvh).view(np.uint8),
                        swz(wvl).view(np.uint8)], axis=1)
        im = {
            "xh": xhs[b],
            "xl": xls[b],
            "onesv": onesv,
            "w0": np.ascontiguousarray(wqk[0]),
            "w1": np.ascontiguousarray(wqk[1]),
            "wv": np.ascontiguousarray(wvp),
            "misc": np.ascontiguousarray(misc),
        }
        if with_bias:
            im["bv"] = np.ascontiguousarray(
                (bv[cs:cs + CSL] * WS).reshape(1, -1))
        in_maps.append(im)

    try:
        res = bass_utils.run_bass_kernel_spmd(
            nc, in_maps, core_ids=list(range(N_CORES)), trace=False)
    except Exception:
        # transient axon-worker/NRT failures recover on retry
        res = bass_utils.run_bass_kernel_spmd(
            nc, in_maps, core_ids=list(range(N_CORES)), trace=False)

    y = np.zeros((B, T, C), np.float32)
    for core in range(N_CORES):
        b, hg = core // HPC, core % HPC
        out = res.results[core]["out"]      # [128, nch, ns, HPC, DH+1]
        ix, tv = idxs[b], tvs[b]
        if not tv:
            continue
        split3 = False
        out3 = res.results[core]["out3"]
        for h in range(HPC):
            col = hg * CSL + h * DH
            for j in range(nch):
                if split3 and h == HPC - 1:
                    q0 = j * cw
                    n = min(cw, tv - q0)
                    if n <= 0:
                        continue
                    blk = out3[:, j, 0:n]                # [65, n]
                    y[b, ix[q0:q0 + n], col:col + DH] = (
                        blk[:DH] / blk[DH:DH + 1] / WS).T
                    continue
                for si, (o, w) in enumerate(subs):
                    q0 = j * cw + o
                    n = min(w, tv - q0)
                    if n <= 0:
                        continue
                    blk = out[0:n, j, si, h, :].astype(np.float32)
                    numer = blk[:, :DH]
                    denom = blk[:, DH:DH + 1]
                    y[b, ix[q0:q0 + n], col:col + DH] = (
                        numer / denom / WS)
    return y
